# revision 1
# baseline (speedup 1.0000x reference)
"""Trainium2 SPMD kernel for nn_Attentionlayer_9208409883387.

Mathematical simplification: the reference computes
    h   = x @ W
    att = softmax(mask(leaky_relu(s1+s2), adj), axis=3)
    res = leaky_relu(h * sum_j att[..., j])
The row-sum of a softmax along its normalization axis is identically 1
(every row has >=1 unmasked entry: P[all-zero adj row] ~ 2^-1024), so
    res = leaky_relu(x @ W)
exactly, up to fp32 rounding of the softmax row-sum (measured absmax-
normalized deviation ~7e-7 against the full reference).

Strategy: data-parallel over the 48*1024 = 49152 rows, 6144 rows/core.
Each core's shard is laid out host-side with f_in on partitions
(xpack[0:64] = rows[0:3072].T, xpack[64:128] = rows[3072:6144].T) so the
PE can consume it directly as the moving operand.  W is replicated as a
block-diagonal W (+) W [128,128] stationary operand, so a single
full-array matmul per [128,512] chunk computes both row-blocks.  The
matmul runs in float32r (single-pass PE, 4x the fp32 rate; measured
absmax-normalized error vs the fp32 reference 1.6e-4).  leaky_relu is a
single ACT-engine Lrelu(alpha=0.01) reading PSUM directly.

Measured on trn2 (8 cores, NTFF profile): ~27.6us/core mean (27.1-29.8
worst-core depending on HBM contention noise), of which ~14us is fixed
NEFF preamble/teardown (an empty kernel measures ~19us) and the rest is
the HBM-bound DMA stream (3.2MB/core; inputs on both HWDGE rings
concurrently, ~310-380GB/s) plus two ~1.3us DMA latency bookends, with
matmul/activation fully hidden underneath.
"""

import numpy as np

B, T, N, F = 4, 12, 1024, 64
N_CORES = 8
ROWS = B * T * N              # 49152
RPC = ROWS // N_CORES         # 6144 rows per core
HALF = RPC // 2               # 3072 packed columns per core
CHUNK = 512                   # fp32 moving-operand max / one PSUM bank
NCHUNK = HALF // CHUNK        # 6

_PROGRAM = None


def _build_program_raw():
    """Raw-Bass pipeline: hand-placed semaphores, no Tile/Bacc event-sem
    tail.  Inputs stream on BOTH HWDGE rings concurrently (SP: W+c0,c1
    then c2,c3; ACT ring: c4,c5) which deepens HBM read pipelining and
    ends the in-stream sooner; compute runs in arrival order
    (c0,c1,c4,c5,c2,c3) so the tail posting chain starts earlier.  PE runs
    one fp32r matmul per [128,512] chunk (W stationary), ACT applies
    Lrelu straight out of PSUM, outputs post on the SP ring with
    single-chunk leading outs to minimize the ring-idle gap.  Measured
    ~27.6us mean vs 28.1 for the single-ring in-stream."""
    import concourse.bass as bass
    import concourse.mybir as mybir
    from contextlib import ExitStack

    f32 = mybir.dt.float32
    f32r = mybir.dt.float32r
    nc = bass.Bass("TRN2")
    xp = nc.declare_dram_parameter("xpack", [128, 128 + HALF], f32r, isOutput=False)
    yp = nc.declare_dram_parameter("ypack", [128, HALF], f32, isOutput=True)

    # xpack columns: W 0:128, chunk i at 128+512i.  Compute in arrival
    # order; outs wait on act_sem counts (order below), single-chunk first.
    CHUNK_ORDER = [0, 1, 4, 5, 2, 3]
    OUTS = [(0, 512, 1), (512, 1024, 2), (2048, 3072, 4), (1024, 2048, 6)]

    with ExitStack() as ctx:
        x_sb = ctx.enter_context(nc.sbuf_tensor("x_sb", [128, 128 + HALF], f32r))
        y_sb = ctx.enter_context(nc.sbuf_tensor("y_sb", [128, HALF], f32))
        warm = ctx.enter_context(nc.sbuf_tensor("warm", [1, 4], f32))
        ps = [
            ctx.enter_context(nc.psum_tensor(f"ps{i}", [128, CHUNK], f32))
            for i in range(NCHUNK)
        ]
        # One semaphore per input DMA: a shared counter would count the 16
        # per-SDMA-engine sub-completions of DIFFERENT transfers together.
        dinA = ctx.enter_context(nc.semaphore("dinA"))
        dinB = ctx.enter_context(nc.semaphore("dinB"))
        dinC = ctx.enter_context(nc.semaphore("dinC"))
        chunk_sem = {0: dinA, 1: dinA, 4: dinB, 5: dinB, 2: dinC, 3: dinC}
        pe_sem = ctx.enter_context(nc.semaphore("pe_sem"))
        act_sem = ctx.enter_context(nc.semaphore("act_sem"))
        dma_out = ctx.enter_context(nc.semaphore("dma_out"))
        block = ctx.enter_context(nc.Block())

        @block.sync
        def _(sync):
            sync.dma_start(out=x_sb[:, 0:1152], in_=xp[:, 0:1152]).then_inc(dinA, 16)
            sync.dma_start(out=x_sb[:, 1152:2176], in_=xp[:, 1152:2176]).then_inc(dinC, 16)
            for lo, hi, need in OUTS:
                sync.wait_ge(act_sem, need)
                sync.dma_start(out=yp[:, lo:hi], in_=y_sb[:, lo:hi]).then_inc(dma_out, 16)
            sync.wait_ge(dma_out, 16 * len(OUTS))

        @block.tensor
        def _(tensor):
            w_ap = x_sb[:, 0:128]
            waited = set()
            for i in CHUNK_ORDER:
                s = chunk_sem[i]
                if s.name not in waited:
                    tensor.wait_ge(s, 16)
                    waited.add(s.name)
                nc.tensor.matmul(
                    ps[i][:],
                    w_ap,
                    x_sb[:, 128 + i * CHUNK : 128 + (i + 1) * CHUNK],
                    start=True,
                    stop=True,
                ).then_inc(pe_sem, 1)

        @block.scalar
        def _(scalar):
            # c4,c5 stream on the ACT HWDGE ring, concurrent with SP's reads.
            scalar.dma_start(
                out=x_sb[:, 2176:3200], in_=xp[:, 2176:3200]
            ).then_inc(dinB, 16)
            # Touch the Lrelu table so walrus's lazy ACT_TABLE_LOAD (~1.3us)
            # runs during the DMA preamble, not before the first real ACT.
            nc.scalar.activation(
                warm[:, :], warm[:, :],
                mybir.ActivationFunctionType.Lrelu, alpha=0.01,
            )
            for k, i in enumerate(CHUNK_ORDER):
                scalar.wait_ge(pe_sem, k + 1)
                nc.scalar.activation(
                    y_sb[:, i * CHUNK : (i + 1) * CHUNK],
                    ps[i][:],
                    mybir.ActivationFunctionType.Lrelu,
                    alpha=0.01,
                ).then_inc(act_sem, 1)

    nc.finalize()
    return nc


def _build_program_tile():
    import concourse.bacc as bacc
    import concourse.mybir as mybir
    from concourse.tile import TileContext

    f32 = mybir.dt.float32
    f32r = mybir.dt.float32r
    # Bacc (not plain Bass): its compile() legalizes multi-semaphore waits
    # (event semaphores, matmul-wait hoisting) that walrus codegen cannot
    # encode directly.
    nc = bacc.Bacc("TRN2")
    # xpack[:, 0:128] is the block-diagonal W; x data starts at column 128.
    # W rides in the same DMA as chunk 0.  The x/W chain is typed float32r
    # so the PE runs single-pass (4x faster than exact fp32).
    xp = nc.declare_dram_parameter("xpack", [128, 128 + HALF], f32r, isOutput=False)
    yp = nc.declare_dram_parameter("ypack", [128, HALF], f32, isOutput=True)

    with TileContext(nc) as tc:
        with (
            tc.tile_pool(name="wxpool", bufs=1) as wxpool,
            tc.tile_pool(name="xpool", bufs=NCHUNK - 1) as xpool,
            tc.tile_pool(name="pspool", bufs=NCHUNK, space="PSUM") as pspool,
            tc.tile_pool(name="ypool", bufs=NCHUNK) as ypool,
        ):
            wx_sb = wxpool.tile([128, 128 + CHUNK], f32r)
            nc.sync.dma_start(out=wx_sb[:], in_=xp[:, 0 : 128 + CHUNK])
            w_sb = wx_sb[:, 0:128]
            for i in range(NCHUNK):
                if i == 0:
                    x_sb = wx_sb[:, 128 : 128 + CHUNK]
                else:
                    x_tile = xpool.tile([128, CHUNK], f32r)
                    nc.sync.dma_start(
                        out=x_tile[:],
                        in_=xp[:, 128 + i * CHUNK : 128 + (i + 1) * CHUNK],
                    )
                    x_sb = x_tile[:]
                z_ps = pspool.tile([128, CHUNK], f32)
                nc.tensor.matmul(
                    z_ps[:],
                    w_sb,
                    x_sb,
                    start=True,
                    stop=True,
                )
                y_sb = ypool.tile([128, CHUNK], f32)
                nc.scalar.activation(
                    y_sb[:],
                    z_ps[:],
                    mybir.ActivationFunctionType.Lrelu,
                    alpha=0.01,
                )
                nc.sync.dma_start(
                    out=yp[:, i * CHUNK : (i + 1) * CHUNK], in_=y_sb[:]
                )
    nc.finalize()
    return nc


_build_program = _build_program_raw


def _get_program():
    global _PROGRAM
    if _PROGRAM is None:
        _PROGRAM = _build_program()
    return _PROGRAM


def _make_in_maps(x, W):
    xr = np.ascontiguousarray(x, dtype=np.float32).reshape(N_CORES, RPC, F)
    wpack = np.zeros((128, 128), np.float32)
    wpack[0:64, 0:64] = W
    wpack[64:128, 64:128] = W
    in_maps = []
    for c in range(N_CORES):
        xpack = np.empty((128, 128 + HALF), np.float32)
        xpack[:, 0:128] = wpack
        xpack[0:64, 128:] = xr[c, 0:HALF].T
        xpack[64:128, 128:] = xr[c, HALF:].T
        in_maps.append({"xpack": xpack})
    return in_maps


def run_spmd(x, W, **spmd_kwargs):
    """Run the Bass program on 8 cores; returns (y_full, BassKernelResults)."""
    from concourse.bass_utils import run_bass_kernel_spmd

    in_maps = _make_in_maps(x, W)
    res = run_bass_kernel_spmd(
        _get_program(), in_maps, list(range(N_CORES)), **spmd_kwargs
    )
    y = np.empty((N_CORES, RPC, F), np.float32)
    for c in range(N_CORES):
        ypack = np.asarray(res.results[c]["ypack"])
        y[c, 0:HALF] = ypack[0:64].T
        y[c, HALF:] = ypack[64:128].T
    return y.reshape(B, T, N, F), res


def kernel(x, adj, W, a):
    # adj and a are mathematically dead (softmax row-sum == 1); see module doc.
    y, _ = run_spmd(np.asarray(x), np.asarray(W, dtype=np.float32))
    return y

